# revision 6
# baseline (speedup 1.0000x reference)
"""GATv2 layer (global-edge-softmax variant) as a Bass/Tile SPMD kernel on 8 trn2 cores.

Math (reference semantics):
    h = (x @ W).reshape(N, H, D)
    score[e] = lrelu(h[src[e]]) . a_src + lrelu(h[dst[e]]) . a_dst     (per head)
    w = softmax over ALL edges (incl. self loops) of score
    out[n] = sum_{e: dst[e]=n} w[e] * h[src[e]]  (+ bias)

Key decomposition: score[e] = as[src[e]] + ad[dst[e]] where
    as[n,h] = lrelu(h[n,h,:]) . a_src[h],  ad[n,h] = lrelu(h[n,h,:]) . a_dst[h]
so with eas = exp(as), ead = exp(ad), Z_h = sum_e exp(score[e,h]):
    out[n] = (ead[n]/Z) * sum_{e: dst[e]=n} eas[src[e]] * h[src[e]]
The inner sum is an unweighted segment-sum of per-node rows
    G[n] = [eas[n] (*) h[n]  |  eas[n]  |  ead[n]]
and Z_h = sum_n ead[n,h] * S[n, HD+h] where S = segment-summed G.

Sharding: nodes are split 8 ways; each core builds its G shard, an AllGather
replicates G; edges are bucketed host-side by dst shard (plus 128-node block
within the shard), so each core's segment-sums are complete for its nodes and
only a 4-float AllReduce (Z) crosses cores. Edge rows are fetched with
indirect (gather) DMA; the per-128-node-block segment-sum is a one-hot matmul
accumulated in PSUM.
"""

import math

import numpy as np

import concourse.bacc as bacc
import concourse.bass as bass
import concourse.mybir as mybir
import concourse.tile as tile
from concourse.bass_utils import run_bass_kernel_spmd

P = 128
CORES = 8
NEG_SLOPE = 0.2


def _cdiv(a, b):
    return (a + b - 1) // b


def _prep_edges(src, dst, N, NS, T_pad_to=None):
    """Bucket edges by (dst core-shard, 128-node block), pad each block's edge
    list to a common per-block chunk count across cores (SPMD needs one
    program). Returns per-core [P, T] index/dst-local arrays + block meta."""
    NB = _cdiv(NS, P)
    order = np.argsort(dst, kind="stable")
    ds_, ss_ = dst[order], src[order]

    cnt = np.zeros((CORES, NB), np.int64)
    segs = {}
    for c in range(CORES):
        for b in range(NB):
            lo = c * NS + b * P
            hi = c * NS + min((b + 1) * P, NS)
            i0 = np.searchsorted(ds_, lo, "left")
            i1 = np.searchsorted(ds_, hi, "left")
            cnt[c, b] = i1 - i0
            segs[(c, b)] = (ss_[i0:i1], (ds_[i0:i1] - lo).astype(np.float32))

    K = np.maximum(1, _cdiv(cnt.max(axis=0), P)).astype(np.int64)  # chunks/block
    CH = np.concatenate([[0], np.cumsum(K)]).astype(np.int64)
    T = int(CH[-1])
    if T_pad_to is not None and T < T_pad_to:
        T = T_pad_to

    src_idx = np.zeros((CORES, T, P), np.int32)  # pad idx 0 (valid row)
    dstl = np.full((CORES, T, P), -1.0, np.float32)  # pad dst -1 (matches no col)
    for c in range(CORES):
        for b in range(NB):
            s_ids, dl = segs[(c, b)]
            n = len(s_ids)
            kb = int(K[b])
            buf_s = np.zeros(kb * P, np.int32)
            buf_d = np.full(kb * P, -1.0, np.float32)
            buf_s[:n] = s_ids
            buf_d[:n] = dl
            j0 = int(CH[b])
            src_idx[c, j0 : j0 + kb] = buf_s.reshape(kb, P)
            dstl[c, j0 : j0 + kb] = buf_d.reshape(kb, P)

    # [C, P, T]: column j = chunk j, partition p = p-th edge of the chunk
    src_idx_T = np.ascontiguousarray(src_idx.transpose(0, 2, 1))
    dstl_T = np.ascontiguousarray(dstl.transpose(0, 2, 1))
    return src_idx_T, dstl_T, [int(k) for k in K], [int(x) for x in CH], T, NB


def _build_program(N, NS, NB, IN_F, HD, H, K, CH, T, GS=4):
    D = HD // H
    GC = HD + 2 * H  # g | eas | ead   (264 for the real problem)
    KT = IN_F // P
    dt = mybir.dt
    f32 = dt.float32
    Alu = mybir.AluOpType
    Act = mybir.ActivationFunctionType

    nc = bacc.Bacc("TRN2", target_bir_lowering=False, debug=False, num_devices=CORES)

    xT = nc.dram_tensor("xT", [IN_F, NS], f32, kind="ExternalInput")
    W_ = nc.dram_tensor("W", [IN_F, HD], f32, kind="ExternalInput")
    asrc = nc.dram_tensor("asrc", [P, HD], f32, kind="ExternalInput")
    adst = nc.dram_tensor("adst", [P, HD], f32, kind="ExternalInput")
    bias = nc.dram_tensor("bias", [P, HD], f32, kind="ExternalInput")
    sidx = nc.dram_tensor("sidx", [P, T], dt.int32, kind="ExternalInput")
    dstl = nc.dram_tensor("dstl", [P, T], f32, kind="ExternalInput")
    out = nc.dram_tensor("out", [NS, HD], f32, kind="ExternalOutput")

    Gsh = nc.dram_tensor("Gsh", [NS, GC], f32)
    Gfull = nc.dram_tensor("Gfull", [N, GC], f32)
    zin = nc.dram_tensor("zin", [1, 8], f32)
    zout = nc.dram_tensor("zout", [1, 8], f32, addr_space="Shared")

    with tile.TileContext(nc) as tc:
        with (
            tc.tile_pool(name="const", bufs=1) as cp,
            tc.tile_pool(name="work", bufs=3) as wk,
            tc.tile_pool(name="gath", bufs=4) as gp,
            tc.tile_pool(name="ohp", bufs=4) as ohp,
            tc.tile_pool(name="pers", bufs=1) as pers,
            tc.tile_pool(name="ps", bufs=2, space="PSUM") as ps,
            tc.tile_pool(name="psS", bufs=2, space="PSUM") as psS,
        ):
            # ---- constants ----
            W_sb = cp.tile([P, KT * HD], f32)
            for kk in range(KT):
                nc.sync.dma_start(
                    out=W_sb[:, kk * HD : (kk + 1) * HD],
                    in_=W_[kk * P : (kk + 1) * P, :],
                )
            asrc_sb = cp.tile([P, HD], f32)
            nc.sync.dma_start(out=asrc_sb[:], in_=asrc[:, :])
            adst_sb = cp.tile([P, HD], f32)
            nc.sync.dma_start(out=adst_sb[:], in_=adst[:, :])
            bias_sb = cp.tile([P, HD], f32)
            nc.sync.dma_start(out=bias_sb[:], in_=bias[:, :])
            sidx_sb = cp.tile([P, T], dt.int32)
            nc.sync.dma_start(out=sidx_sb[:], in_=sidx[:, :])
            dstl_sb = cp.tile([P, T], f32)
            nc.sync.dma_start(out=dstl_sb[:], in_=dstl[:, :])

            iota_i = cp.tile([P, P], dt.int32)
            nc.gpsimd.iota(iota_i[:], pattern=[[1, P]], base=0, channel_multiplier=0)
            iota_f = cp.tile([P, P], f32)
            nc.vector.tensor_copy(out=iota_f[:], in_=iota_i[:])
            ones_col = cp.tile([P, 1], f32)
            nc.vector.memset(ones_col[:], 1.0)
            ones_row = cp.tile([1, P], f32)
            nc.vector.memset(ones_row[:], 1.0)

            # ---- persistent accumulators ----
            ead_all = pers.tile([P, NB * H], f32)
            z_all = pers.tile([P, NB * H], f32)
            s_all = pers.tile([P, NB * HD], f32)

            # ---- phase 1: build G shard ----
            for b in range(NB):
                wb = min(P, NS - b * P)
                ph = ps.tile([P, HD], f32, tag="ph")
                xt = wk.tile([P, KT * P], f32, tag="xt")
                for kk in range(KT):
                    nc.sync.dma_start(
                        out=xt[:, kk * P : kk * P + wb],
                        in_=xT[kk * P : (kk + 1) * P, b * P : b * P + wb],
                    )
                for kk in range(KT):
                    nc.tensor.matmul(
                        out=ph[:wb, :],
                        lhsT=xt[:, kk * P : kk * P + wb],
                        rhs=W_sb[:, kk * HD : (kk + 1) * HD],
                        start=(kk == 0),
                        stop=(kk == KT - 1),
                    )
                # leaky relu = max(v, slope*v) — ACT Lrelu is not sim-supported
                lr = wk.tile([P, HD], f32, tag="lr")
                nc.vector.tensor_scalar(
                    out=lr[:wb, :], in0=ph[:wb, :], scalar1=NEG_SLOPE,
                    scalar2=None, op0=Alu.mult,
                )
                nc.vector.tensor_tensor(
                    out=lr[:wb, :], in0=ph[:wb, :], in1=lr[:wb, :], op=Alu.max
                )
                easr = wk.tile([P, H], f32, tag="easr")
                t1 = wk.tile([P, HD], f32, tag="t1")
                nc.vector.tensor_tensor(
                    out=t1[:wb, :], in0=lr[:wb, :], in1=asrc_sb[:wb, :], op=Alu.mult
                )
                ar = wk.tile([P, H], f32, tag="ar")
                nc.vector.tensor_reduce(
                    out=ar[:wb, :],
                    in_=t1[:wb, :].rearrange("p (h d) -> p h d", h=H),
                    axis=mybir.AxisListType.X,
                    op=Alu.add,
                )
                nc.scalar.activation(out=easr[:wb, :], in_=ar[:wb, :], func=Act.Exp)
                t2 = wk.tile([P, HD], f32, tag="t2")
                nc.vector.tensor_tensor(
                    out=t2[:wb, :], in0=lr[:wb, :], in1=adst_sb[:wb, :], op=Alu.mult
                )
                ar2 = wk.tile([P, H], f32, tag="ar2")
                nc.vector.tensor_reduce(
                    out=ar2[:wb, :],
                    in_=t2[:wb, :].rearrange("p (h d) -> p h d", h=H),
                    axis=mybir.AxisListType.X,
                    op=Alu.add,
                )
                if wb < P:
                    # full-column memset first (engines need 32-aligned base
                    # partitions, so no [wb:] slice); exp overwrites [:wb]
                    nc.vector.memset(ead_all[:, b * H : (b + 1) * H], 0.0)
                nc.scalar.activation(
                    out=ead_all[:wb, b * H : (b + 1) * H], in_=ar2[:wb, :], func=Act.Exp
                )
                g = wk.tile([P, GC], f32, tag="g")
                for h in range(H):
                    nc.vector.tensor_scalar(
                        out=g[:wb, h * D : (h + 1) * D],
                        in0=ph[:wb, h * D : (h + 1) * D],
                        scalar1=easr[:wb, h : h + 1],
                        scalar2=None,
                        op0=Alu.mult,
                    )
                nc.vector.tensor_copy(out=g[:wb, HD : HD + H], in_=easr[:wb, :])
                nc.vector.tensor_copy(
                    out=g[:wb, HD + H : GC], in_=ead_all[:wb, b * H : (b + 1) * H]
                )
                nc.sync.dma_start(out=Gsh[b * P : b * P + wb, :], in_=g[:wb, :])

            # ---- replicate G ----
            nc.gpsimd.collective_compute(
                "AllGather",
                Alu.bypass,
                replica_groups=[list(range(CORES))],
                ins=[Gsh.ap().opt()],
                outs=[Gfull.ap().opt()],
            )

            # ---- phase 2: edge segment-sums (per 128-node block) ----
            # NOTE: indirect DMA gather only honors a single index column on
            # HW (multi-column index APs read contiguous rows from idx[:,0]),
            # so it's one gather instruction per 128-edge chunk.
            for b in range(NB):
                Kb, j0 = K[b], CH[b]
                S = psS.tile([P, GC], f32, tag="S")
                for k in range(Kb):
                    j = j0 + k
                    gt = gp.tile([P, GC], f32, tag="gt")
                    nc.gpsimd.indirect_dma_start(
                        out=gt[:, :],
                        out_offset=None,
                        in_=Gfull[:, :],
                        in_offset=bass.IndirectOffsetOnAxis(
                            ap=sidx_sb[:, j : j + 1], axis=0
                        ),
                    )
                    oh = ohp.tile([P, P], f32, tag="oh")
                    nc.vector.tensor_scalar(
                        out=oh[:],
                        in0=iota_f[:],
                        scalar1=dstl_sb[:, j : j + 1],
                        scalar2=None,
                        op0=Alu.is_equal,
                    )
                    nc.tensor.matmul(
                        out=S[:],
                        lhsT=oh[:],
                        rhs=gt[:, :],
                        start=(j == j0),
                        stop=(j == j0 + Kb - 1),
                    )
                nc.vector.tensor_tensor(
                    out=z_all[:, b * H : (b + 1) * H],
                    in0=ead_all[:, b * H : (b + 1) * H],
                    in1=S[:, HD : HD + H],
                    op=Alu.mult,
                )
                nc.any.tensor_copy(out=s_all[:, b * HD : (b + 1) * HD], in_=S[:, :HD])

            # ---- Z: partition-reduce, AllReduce, reciprocal, broadcast ----
            zp = ps.tile([1, NB * H], f32, tag="zp")
            nc.tensor.matmul(
                out=zp[:], lhsT=ones_col[:], rhs=z_all[:], start=True, stop=True
            )
            z4 = wk.tile([1, 8], f32, tag="z4")
            nc.vector.memset(z4[:], 0.0)
            nc.vector.tensor_reduce(
                out=z4[:, :H],
                in_=zp[:].rearrange("p (b h) -> p h b", h=H),
                axis=mybir.AxisListType.X,
                op=Alu.add,
            )
            nc.sync.dma_start(out=zin[:, :], in_=z4[:])
            nc.gpsimd.collective_compute(
                "AllReduce",
                Alu.add,
                replica_groups=[list(range(CORES))],
                ins=[zin.ap().opt()],
                outs=[zout.ap().opt()],
            )
            zg = wk.tile([1, 8], f32, tag="zg")
            nc.sync.dma_start(out=zg[:], in_=zout[:, :])
            zb_ps = ps.tile([P, H], f32, tag="zb")
            nc.tensor.matmul(
                out=zb_ps[:], lhsT=ones_row[:], rhs=zg[:, :H], start=True, stop=True
            )
            zb = wk.tile([P, H], f32, tag="zbs")
            nc.vector.tensor_copy(out=zb[:], in_=zb_ps[:])
            invz = wk.tile([P, H], f32, tag="invz")
            nc.vector.reciprocal(out=invz[:], in_=zb[:])

            # ---- finalize: out = s * (ead/Z) + bias ----
            for b in range(NB):
                wb = min(P, NS - b * P)
                ft = wk.tile([P, H], f32, tag="ft")
                nc.vector.tensor_tensor(
                    out=ft[:],
                    in0=ead_all[:, b * H : (b + 1) * H],
                    in1=invz[:],
                    op=Alu.mult,
                )
                ob = wk.tile([P, HD], f32, tag="ob")
                for h in range(H):
                    nc.vector.tensor_scalar(
                        out=ob[:, h * D : (h + 1) * D],
                        in0=s_all[:, b * HD + h * D : b * HD + (h + 1) * D],
                        scalar1=ft[:, h : h + 1],
                        scalar2=None,
                        op0=Alu.mult,
                    )
                nc.vector.tensor_tensor(
                    out=ob[:], in0=ob[:], in1=bias_sb[:], op=Alu.add
                )
                nc.sync.dma_start(out=out[b * P : b * P + wb, :], in_=ob[:wb, :])

    nc.compile()
    return nc


def make_in_maps(x, W, att, b, edge_index):
    """Host-side sharding: returns (in_maps, meta) for the SPMD program."""
    N, IN_F = x.shape
    HD = W.shape[1]
    H = att.shape[1]
    E = edge_index.shape[1]
    assert N % CORES == 0
    NS = N // CORES

    loops = np.arange(N, dtype=np.int64)
    src = np.concatenate([np.asarray(edge_index[0]), loops]).astype(np.int64)
    dst = np.concatenate([np.asarray(edge_index[1]), loops]).astype(np.int64)

    src_idx_T, dstl_T, K, CH, T, NB = _prep_edges(src, dst, N, NS)

    x = np.asarray(x, np.float32)
    W = np.asarray(W, np.float32)
    att = np.asarray(att, np.float32)
    b = np.asarray(b, np.float32)
    D = att.shape[2] // 2
    a_src = np.tile(att[0, :, :D].reshape(1, H * D), (P, 1)).astype(np.float32)
    a_dst = np.tile(att[0, :, D:].reshape(1, H * D), (P, 1)).astype(np.float32)
    bias = np.tile(b.reshape(1, HD), (P, 1)).astype(np.float32)

    in_maps = []
    for c in range(CORES):
        xT_sh = np.ascontiguousarray(x[c * NS : (c + 1) * NS, :].T)
        in_maps.append(
            {
                "xT": xT_sh,
                "W": W,
                "asrc": a_src,
                "adst": a_dst,
                "bias": bias,
                "sidx": src_idx_T[c],
                "dstl": dstl_T[c],
            }
        )
    meta = dict(N=N, NS=NS, NB=NB, IN_F=IN_F, HD=HD, H=H, K=K, CH=CH, T=T)
    return in_maps, meta


_PROGRAM_CACHE = {}


def kernel(x, W, att, b, edge_index):
    in_maps, meta = make_in_maps(x, W, att, b, edge_index)
    key = (meta["N"], meta["IN_F"], meta["HD"], meta["H"], meta["T"], tuple(meta["K"]))
    nc = _PROGRAM_CACHE.get(key)
    if nc is None:
        nc = _build_program(
            meta["N"], meta["NS"], meta["NB"], meta["IN_F"], meta["HD"], meta["H"],
            meta["K"], meta["CH"], meta["T"],
        )
        _PROGRAM_CACHE[key] = nc
    res = run_bass_kernel_spmd(nc, in_maps, core_ids=list(range(CORES)))
    out = np.concatenate([res.results[c]["out"] for c in range(CORES)], axis=0)
    return out.astype(np.float32)
